# revision 1
# baseline (speedup 1.0000x reference)
"""Trainium2 Bass kernel for nn_DualBranchDecoder.

Dual-branch band-split decoder: per-band GroupNorm -> fc1(C=128->H=512)+tanh
-> per-band fc2(H->w_k) -> sigmoid mag mask / tanh phase offset -> complex out.

Sharding: data-parallel over batch B=8 across 8 NeuronCores (one sample per
core).

Schedule: the per-quad GroupNorm stats chain (DVE bn_stats -> PE cross-
partition sum -> DVE rsqrt -> PE broadcast) is software-pipelined one quad
ahead, with the two tiny PE reductions emitted mid-way through the current
quad's matmul stream at points where their DVE inputs are already complete —
so neither engine's in-order queue ever stalls at a quad boundary. The mag
sigmoid is computed as tanh (0.5s folded into host W2/b2, +1/2 into the
final mask multiply), keeping activations on the single {tanh,sin} table;
the fin sin/cos run once at the end. The single f=256 output row is folded
into fin chunk 1 as 4 extra columns via reshaping DMAs.
"""
import sys
sys.path.insert(0, '/opt/trn_rl_repo')

import numpy as np

import concourse.bacc as bacc
import concourse.tile as tile
import concourse.mybir as mybir
from concourse.bass_utils import run_bass_kernel_spmd

F32 = mybir.dt.float32
FP16 = mybir.dt.float16
H1DT = FP16
W2DT = FP16
AF = mybir.ActivationFunctionType
ALU = mybir.AluOpType

# problem constants (hardcoded per contract)
B, C, T = 8, 128, 512
BANDS = [2] + [3] * 10 + [8] * 12 + [16] * 7 + [17]
K = len(BANDS)                      # 31
F = sum(BANDS)                      # 257
H = 4 * C                           # 512
NHC = H // 128                      # 4 h-chunks
EPS = 1e-5

OFFS = np.concatenate([[0], np.cumsum(BANDS)]).astype(int)   # band start freqs
WPADS = [w + (w & 1) for w in BANDS]
WOFFS = np.concatenate([[0], np.cumsum(WPADS)]).astype(int)
WPTOT = int(WOFFS[-1])

QUADS = [(4 * i, 4) for i in range(7)] + [(28, 3)]
NQ = len(QUADS)
MAGIC = float(1.5 * 2 ** 23)
INV2PI = float(1.0 / (2 * np.pi))
N2PI = float(-2 * np.pi)
PI = float(np.pi)
TE = T + 4                          # fin chunk-1 width (f=256 row folded in)
SIN_WIDE = False                    # Sin table only covers ~[-pi,3pi/2]; keep
                                    # the magic-rounding range reduction

_cache = {}


def _prep_branch(gamma, beta, W1, b1, W2, b2):
    """Host-side constant prep for one branch. W2/b2 must be pre-scaled by
    the caller if the branch folds sigmoid into tanh."""
    W1g = W1 * gamma[:, None, :]                      # [K, H, C]
    W1gT = np.ascontiguousarray(W1g.transpose(2, 0, 1).reshape(C, K * H))
    W1gT = W1gT.astype(np.float16)
    b1p = b1 + np.einsum('khc,kc->kh', W1, beta)      # [K, H]
    b1pT = np.zeros((128, K * NHC), np.float32)
    for k in range(K):
        for hc in range(NHC):
            b1pT[:, k * NHC + hc] = b1p[k, hc * 128:(hc + 1) * 128]
    W2Tp = np.zeros((128, NHC * WPTOT), np.float32)
    for k in range(K):
        w, off, woff = BANDS[k], OFFS[k], WOFFS[k]
        for hc in range(NHC):
            W2Tp[:, hc * WPTOT + woff: hc * WPTOT + woff + w] = \
                W2[off:off + w, hc * 128:(hc + 1) * 128].T
    W2Tp = W2Tp.astype(np.float16)
    b2g = np.zeros((128, NQ), np.float32)
    for q, (k0, nb) in enumerate(QUADS):
        for r in range(nb):
            k = k0 + r
            b2g[32 * r:32 * r + BANDS[k], q] = b2[OFFS[k]:OFFS[k] + BANDS[k]]
    return W1gT, b1pT, W2Tp, b2g


def _build():
    nc = bacc.Bacc("TRN2", target_bir_lowering=False)

    ins = {}
    for br in ("m", "p"):
        ins[f"feat_{br}"] = nc.dram_tensor(f"feat_{br}", [C, K * T], F32,
                                           kind="ExternalInput")
        ins[f"w1gt_{br}"] = nc.dram_tensor(f"w1gt_{br}", [C, K * H], FP16,
                                           kind="ExternalInput")
        ins[f"b1pt_{br}"] = nc.dram_tensor(f"b1pt_{br}", [128, K * NHC], F32,
                                           kind="ExternalInput")
        ins[f"w2tp_{br}"] = nc.dram_tensor(f"w2tp_{br}", [128, NHC * WPTOT], W2DT,
                                           kind="ExternalInput")
        ins[f"b2c_{br}"] = nc.dram_tensor(f"b2c_{br}", [128, NQ], F32,
                                          kind="ExternalInput")
        ins[f"noisy_{br}"] = nc.dram_tensor(f"noisy_{br}", [F, T], F32,
                                            kind="ExternalInput")
    ones_col_d = nc.dram_tensor("ones_col", [128, 1], F32, kind="ExternalInput")
    ones_row_d = nc.dram_tensor("ones_row", [1, 128], F32, kind="ExternalInput")
    halfpi_d = nc.dram_tensor("halfpi", [128, 1], F32, kind="ExternalInput")
    out_d = nc.dram_tensor("out", [F, 2 * T], F32, kind="ExternalOutput")

    with tile.TileContext(nc) as tc:
        with (
            tc.tile_pool(name="featk", bufs=3) as featk_pool,
            tc.tile_pool(name="w1t", bufs=3) as w1t_pool,
            tc.tile_pool(name="h1sb", bufs=3) as h1sb_pool,
            tc.tile_pool(name="band", bufs=4) as band_pool,
            tc.tile_pool(name="const", bufs=1) as const_pool,
            tc.tile_pool(name="statsb", bufs=2) as stats_pool,
            tc.tile_pool(name="fin", bufs=2) as fin_pool,
            tc.tile_pool(name="mainps", bufs=1, space="PSUM") as main_ps,
        ):
            # ---- critical-path first: quad-0 fetches before anything else ----
            st = {}     # per-quad pipeline state
            k0_0, nb_0 = QUADS[0]
            st[0] = {}
            st[0]["fq_m"] = featk_pool.tile([128, nb_0 * T], F32, tag="featq",
                                            name="featq_m_0")
            nc.sync.dma_start(st[0]["fq_m"][:],
                              ins["feat_m"][:, k0_0 * T:(k0_0 + nb_0) * T])
            st[0]["wq_m"] = w1t_pool.tile([128, nb_0 * H], FP16, tag="w1q",
                                          name="w1q_m_0")
            nc.sync.dma_start(st[0]["wq_m"][:],
                              ins["w1gt_m"][:, k0_0 * H:(k0_0 + nb_0) * H])
            st[0]["fq_p"] = featk_pool.tile([128, nb_0 * T], F32, tag="featq",
                                            name="featq_p_0")
            nc.sync.dma_start(st[0]["fq_p"][:],
                              ins["feat_p"][:, k0_0 * T:(k0_0 + nb_0) * T])

            # ---- constants ----
            ones_col = const_pool.tile([128, 1], F32)
            nc.sync.dma_start(ones_col[:], ones_col_d[:])
            ones_row = const_pool.tile([1, 128], F32)
            nc.sync.dma_start(ones_row[:], ones_row_d[:])
            halfpi = const_pool.tile([128, 1], F32)
            nc.sync.dma_start(halfpi[:], halfpi_d[:])

            cb = {}
            for br in ("m", "p"):
                b1pt = const_pool.tile([128, K * NHC], F32, tag=f"b1pt_{br}",
                                       name=f"b1pt_{br}")
                nc.sync.dma_start(b1pt[:], ins[f"b1pt_{br}"][:])
                w2tp = const_pool.tile([128, NHC * WPTOT], W2DT, tag=f"w2tp_{br}",
                                       name=f"w2tp_{br}")
                nc.sync.dma_start(w2tp[:], ins[f"w2tp_{br}"][:])
                b2c = const_pool.tile([128, NQ], F32, tag=f"b2c_{br}",
                                      name=f"b2c_{br}")
                nc.sync.dma_start(b2c[:], ins[f"b2c_{br}"][:])
                cb[br] = (b1pt, w2tp, b2c)

            st[0]["wq_p"] = w1t_pool.tile([128, nb_0 * H], FP16, tag="w1q",
                                          name="w1q_p_0")
            nc.sync.dma_start(st[0]["wq_p"][:],
                              ins["w1gt_p"][:, k0_0 * H:(k0_0 + nb_0) * H])

            # noisy inputs prefetched into per-chunk tiles for the fin stage.
            # Chunk 1 is [128, T+4]: cols T.. hold the f=256 row reshaped as
            # [128, 4] (t = 4*p + c), so no third full-width pipeline exists.
            noisy = {}
            for br in ("m", "p"):
                n0 = const_pool.tile([128, T], F32, tag=f"noisy_{br}_0",
                                     name=f"noisy_{br}_0")
                nc.gpsimd.dma_start(n0[:], ins[f"noisy_{br}"][0:128, :])
                n1 = const_pool.tile([128, TE], F32, tag=f"noisy_{br}_1",
                                     name=f"noisy_{br}_1")
                nc.gpsimd.dma_start(n1[:, 0:T], ins[f"noisy_{br}"][128:256, :])
                nc.gpsimd.dma_start(n1[:, T:TE], ins[f"noisy_{br}"][256:257, :])
                noisy[br, 0] = n0
                noisy[br, 1] = n1

            # ---- PE warm-up: continuous matmul during DMA ramp-in ----
            for wi in range(16):
                wps = main_ps.tile([128, T], F32, tag="h1ps", bufs=5,
                                   name=f"warm_{wi}")
                nc.tensor.matmul(wps[:], st[0]["wq_m"][:, 0:128],
                                 st[0]["wq_m"][:, 0:T], start=True, stop=True)

            masks = {}
            for br in ("m", "p"):
                masks[br, 0] = const_pool.tile([128, T], F32, tag=f"mask_{br}_0",
                                               name=f"mask_{br}_0")
                masks[br, 1] = const_pool.tile([128, TE], F32, tag=f"mask_{br}_1",
                                               name=f"mask_{br}_1")

            # ---------------- pipeline stage emitters ----------------
            def do_dma(q):
                k0, nb = QUADS[q]
                s = st.setdefault(q, {})
                for br in ("m", "p"):
                    s[f"fq_{br}"] = featk_pool.tile([128, nb * T], F32,
                                                    tag="featq",
                                                    name=f"featq_{br}_{q}")
                    nc.sync.dma_start(
                        s[f"fq_{br}"][:],
                        ins[f"feat_{br}"][:, k0 * T:(k0 + nb) * T])
                    s[f"wq_{br}"] = w1t_pool.tile([128, nb * H], FP16,
                                                  tag="w1q",
                                                  name=f"w1q_{br}_{q}")
                    nc.sync.dma_start(
                        s[f"wq_{br}"][:],
                        ins[f"w1gt_{br}"][:, k0 * H:(k0 + nb) * H])

            def do_stats_front(q):
                """bn stats + per-partition sums for both branches (DVE)."""
                k0, nb = QUADS[q]
                nb2 = 2 * nb
                s = st[q]
                st_q = stats_pool.tile([128, nb2 * 6], F32, tag="st_q",
                                       name=f"st_{q}")
                ag_q = stats_pool.tile([128, nb2 * 2], F32, tag="ag_q",
                                       name=f"ag_{q}")
                sums = stats_pool.tile([128, 2 * nb2], F32, tag="sums",
                                       name=f"sums_{q}")
                tmp = stats_pool.tile([128, nb2], F32, tag="tmp", name=f"tmp_{q}")
                for bi, br in enumerate(("m", "p")):
                    for r in range(nb):
                        i = bi * nb + r
                        nc.vector.bn_stats(st_q[:, i * 6:(i + 1) * 6],
                                           s[f"fq_{br}"][:, r * T:(r + 1) * T])
                        nc.vector.bn_aggr(ag_q[:, i * 2:(i + 1) * 2],
                                          st_q[:, i * 6:(i + 1) * 6])
                ag3 = ag_q[:].rearrange("c (k two) -> c k two", two=2)
                nc.vector.tensor_copy(sums[:, 0:nb2], ag3[:, :, 0])
                nc.vector.tensor_mul(tmp[:], ag3[:, :, 0], ag3[:, :, 0])
                nc.vector.tensor_add(sums[:, nb2:2 * nb2], tmp[:], ag3[:, :, 1])
                s["sums"] = sums

            def do_ps_s(q):
                """cross-partition reduction (PE, tiny)."""
                nb2 = 2 * QUADS[q][1]
                s = st[q]
                ps_s = main_ps.tile([1, 2 * nb2], F32, tag="ps_s", bufs=1,
                                    name=f"ps_s_{q}")
                nc.tensor.matmul(ps_s[:], ones_col[:], s["sums"][:],
                                 start=True, stop=True)
                s["ps_s"] = ps_s

            def do_stats_mid(q):
                """mean/var -> rsqrt via quake seed + 3 Newton (DVE)."""
                nb2 = 2 * QUADS[q][1]
                s = st[q]
                g = stats_pool.tile([1, 2 * nb2], F32, tag="g", name=f"g_{q}")
                nc.vector.tensor_scalar_mul(g[:], s["ps_s"][:], 1.0 / C)
                gm2 = stats_pool.tile([1, nb2], F32, tag="gm2", name=f"gm2_{q}")
                nc.vector.tensor_mul(gm2[:], g[:, 0:nb2], g[:, 0:nb2])
                gvar = stats_pool.tile([1, nb2], F32, tag="gvar", name=f"gvar_{q}")
                nc.vector.tensor_sub(gvar[:], g[:, nb2:2 * nb2], gm2[:])
                vv = stats_pool.tile([1, nb2], F32, tag="vv", name=f"vv_{q}")
                nc.vector.tensor_scalar_add(vv[:], gvar[:], EPS)
                I32 = mybir.dt.int32
                yy = stats_pool.tile([1, nb2], F32, tag="yy", name=f"yy_{q}")
                nc.vector.tensor_scalar(yy[:].bitcast(I32), vv[:].bitcast(I32),
                                        1, -1, op0=ALU.arith_shift_right,
                                        op1=ALU.bitwise_xor)
                nc.vector.tensor_scalar_add(yy[:].bitcast(I32),
                                            yy[:].bitcast(I32), 0x5f3759e0)
                invmean = stats_pool.tile([1, 2 * nb2], F32, tag="invmean",
                                          name=f"invmean_{q}")
                tnr = stats_pool.tile([1, nb2], F32, tag="tnr", name=f"tnr_{q}")
                for it in range(3):
                    nc.vector.tensor_mul(tnr[:], yy[:], yy[:])
                    nc.vector.tensor_mul(tnr[:], tnr[:], vv[:])
                    nc.vector.tensor_scalar(tnr[:], tnr[:], -0.5, 1.5,
                                            op0=ALU.mult, op1=ALU.add)
                    dst = yy[:] if it < 2 else invmean[:, 0:nb2]
                    nc.vector.tensor_mul(dst, yy[:], tnr[:])
                nc.vector.tensor_copy(invmean[:, nb2:2 * nb2], g[:, 0:nb2])
                s["invmean"] = invmean

            def do_ps_b(q):
                """broadcast inv/mean to all partitions (PE, tiny)."""
                nb2 = 2 * QUADS[q][1]
                s = st[q]
                ps_b = main_ps.tile([128, 2 * nb2], F32, tag="ps_s", bufs=1,
                                    name=f"ps_b_{q}")
                nc.tensor.matmul(ps_b[:], ones_row[:], s["invmean"][:],
                                 start=True, stop=True)
                s["ps_b"] = ps_b

            def do_bbq(q):
                nb2 = 2 * QUADS[q][1]
                s = st[q]
                bbq = stats_pool.tile([128, 2 * nb2], F32, tag="bbq", bufs=3,
                                      name=f"bbq_{q}")
                nc.vector.tensor_copy(bbq[:], s["ps_b"][:])
                s["bbq"] = bbq
                # bbq[:, bi*nb+r] = inv ; bbq[:, nb2+bi*nb+r] = mean

            def do_fcent(q, bi, br):
                """normalize + cast to fp16: fqh = (x - mean) * inv (DVE)."""
                k0, nb = QUADS[q]
                nb2 = 2 * nb
                s = st[q]
                fq, bbq = s[f"fq_{br}"], s["bbq"]
                fqh = featk_pool.tile([128, nb * T], FP16, tag="fqh",
                                      name=f"fqh_{br}_{q}")
                for r in range(nb):
                    i = bi * nb + r
                    nc.vector.tensor_scalar(
                        fqh[:, r * T:(r + 1) * T],
                        fq[:, r * T:(r + 1) * T],
                        bbq[:, nb2 + i:nb2 + i + 1],
                        bbq[:, i:i + 1],
                        op0=ALU.subtract, op1=ALU.mult)
                s[f"fqh_{br}"] = fqh

            def do_branch(q, br, hook_after_fc2=None, hook_mid_fc1=None):
                """fc1 + fc2 + mask copy-out for one branch of one quad."""
                k0, nb = QUADS[q]
                s = st[q]
                b1pt, w2tp, b2c = cb[br]
                wq, fqh = s[f"wq_{br}"], s[f"fqh_{br}"]
                h1s = []
                for r in range(nb):
                    k = k0 + r
                    h1sb = h1sb_pool.tile([128, NHC * T], H1DT, bufs=6)
                    h1s.append(h1sb)
                    for hc in range(NHC):
                        h1ps = main_ps.tile([128, T], F32, tag="h1ps", bufs=5,
                                            name=f"h1ps_{br}_{k}_{hc}")
                        nc.tensor.matmul(
                            h1ps[:],
                            wq[:, (r * NHC + hc) * 128:(r * NHC + hc + 1) * 128],
                            fqh[:, r * T:(r + 1) * T],
                            start=True, stop=True)
                        nc.scalar.activation(
                            h1sb[:, hc * T:(hc + 1) * T], h1ps[:],
                            AF.Tanh,
                            bias=b1pt[:, k * NHC + hc:k * NHC + hc + 1])
                    if r == 1 and hook_mid_fc1 is not None:
                        hook_mid_fc1()
                fc2g = main_ps.tile([128, T], F32, tag="fc2ps", bufs=2,
                                    name=f"fc2g_{br}_{q}")
                for r in range(nb):
                    k = k0 + r
                    wp, woff = WPADS[k], int(WOFFS[k])
                    for hc in range(NHC):
                        nc.tensor.matmul(
                            fc2g[32 * r:32 * r + wp, :],
                            w2tp[:, hc * WPTOT + woff: hc * WPTOT + woff + wp],
                            h1s[r][:, hc * T:(hc + 1) * T],
                            start=(hc == 0), stop=(hc == NHC - 1),
                            tile_position=(0, 32 * r))
                if hook_after_fc2 is not None:
                    hook_after_fc2()
                grp_t = band_pool.tile([128, T], F32, tag="band")
                # mag: sigmoid(y+b2) = 0.5*(tanh(0.5*y+0.5*b2)+1); the 0.5s
                # live in host W2/b2 prep, the +1/2 in emit_fin.
                nc.scalar.activation(grp_t[:], fc2g[:], AF.Tanh,
                                     bias=b2c[:, q:q + 1])
                dma_eng = nc.sync if q == NQ - 1 else nc.gpsimd
                for r in range(nb):
                    k = k0 + r
                    w, off = BANDS[k], int(OFFS[k])
                    j0, r0 = off // 128, off % 128
                    if off + w <= (j0 + 1) * 128:
                        dma_eng.dma_start(masks[br, j0][r0:r0 + w, 0:T],
                                          grp_t[32 * r:32 * r + w, :])
                    else:
                        n1 = (j0 + 1) * 128 - off
                        dma_eng.dma_start(masks[br, j0][r0:128, 0:T],
                                          grp_t[32 * r:32 * r + n1, :])
                        if j0 == 0:
                            dma_eng.dma_start(
                                masks[br, 1][0:w - n1, 0:T],
                                grp_t[32 * r + n1:32 * r + w, :])
                        else:
                            # f=256 single row -> [128, 4] (t = 4*p + c)
                            dma_eng.dma_start(
                                masks[br, 1][:, T:TE],
                                grp_t[32 * r + n1:32 * r + w, :])

            fin_state = {}

            def emit_fin_pre(j):
                """DVE part of the final assembly for frequency chunk j."""
                cols = T if j == 0 else TE
                mask_ap = masks["m", j][:]
                poff_ap = masks["p", j][:]
                nmag = noisy["m", j]      # pre-halved on host
                nph = noisy["p", j]
                ang = fin_pool.tile([128, cols], F32, tag=f"ang{j}")
                nc.vector.scalar_tensor_tensor(ang[:], poff_ap, PI, nph[:],
                                               op0=ALU.mult, op1=ALU.add)
                enh = fin_pool.tile([128, cols], F32, tag=f"enh{j}")
                nc.vector.scalar_tensor_tensor(enh[:], mask_ap, 1.0, nmag[:],
                                               op0=ALU.add, op1=ALU.mult)
                if not SIN_WIDE:
                    t2 = fin_pool.tile([128, cols], F32, tag=f"t2{j}")
                    nc.vector.tensor_scalar(t2[:], ang[:], INV2PI, MAGIC,
                                            op0=ALU.mult, op1=ALU.add)
                    m2pin = fin_pool.tile([128, cols], F32, tag=f"m2pin{j}")
                    nc.vector.tensor_scalar(m2pin[:], t2[:], MAGIC, N2PI,
                                            op0=ALU.subtract, op1=ALU.mult)
                    nc.vector.tensor_add(m2pin[:], ang[:], m2pin[:])
                    t2c = fin_pool.tile([128, cols], F32, tag=f"t2c{j}")
                    nc.vector.tensor_scalar(t2c[:], ang[:], INV2PI, 0.25,
                                            op0=ALU.mult, op1=ALU.add)
                    nc.vector.tensor_scalar_add(t2c[:], t2c[:], MAGIC)
                    m2pinc = fin_pool.tile([128, cols], F32, tag=f"m2pinc{j}")
                    nc.vector.tensor_scalar(m2pinc[:], t2c[:], MAGIC, N2PI,
                                            op0=ALU.subtract, op1=ALU.mult)
                    nc.vector.tensor_add(m2pinc[:], ang[:], m2pinc[:])
                else:
                    # |ang| <= 2pi; trust the Sin table over the full range
                    m2pin, m2pinc = ang, ang
                fin_state[j] = (cols, enh, m2pin, m2pinc)

            def emit_fin_post(j):
                """Sin/cos + complex assembly + output DMA for chunk j."""
                cols, enh, m2pin, m2pinc = fin_state[j]
                sn = fin_pool.tile([128, cols], F32, tag=f"sn{j}")
                nc.scalar.activation(sn[:], m2pin[:], AF.Sin)
                cn = fin_pool.tile([128, cols], F32, tag=f"cn{j}")
                nc.scalar.activation(cn[:], m2pinc[:], AF.Sin, bias=halfpi[:])
                ot = fin_pool.tile([128, 2 * cols], F32, tag=f"ot{j}")
                ot2 = ot[:].rearrange("p (t two) -> p t two", two=2)
                nc.vector.tensor_mul(ot2[:, :, 0], enh[:], cn[:])
                nc.vector.tensor_mul(ot2[:, :, 1], enh[:], sn[:])
                nc.sync.dma_start(out_d[j * 128:(j + 1) * 128, :], ot[:, 0:2 * T])
                if j == 1:
                    nc.sync.dma_start(out_d[256:257, :], ot[:, 2 * T:2 * TE])

            # ---------------- software-pipelined main loop ----------------
            # prologue: quad-0 stats fully, quad-1 DMA
            do_dma(1)
            do_stats_front(0)
            do_ps_s(0)
            do_stats_mid(0)
            do_ps_b(0)
            do_bbq(0)
            for q in range(NQ):
                if q + 2 < NQ:
                    do_dma(q + 2)
                do_fcent(q, 0, "m")
                do_fcent(q, 1, "p")
                if q + 1 < NQ:
                    do_stats_front(q + 1)
                # stats(q+1) PE reductions are emitted inside the matmul
                # stream at points where their DVE inputs are already done
                do_branch(q, "m",
                          hook_after_fc2=(
                              (lambda qq=q: (do_ps_s(qq + 1),
                                             do_stats_mid(qq + 1)))
                              if q + 1 < NQ else None))
                do_branch(q, "p",
                          hook_mid_fc1=(
                              (lambda qq=q: (do_ps_b(qq + 1),
                                             do_bbq(qq + 1)))
                              if q + 1 < NQ else None))
                if q == 5:
                    # bands 0..22 (f 0..127) complete for both branches
                    emit_fin_pre(0)
            emit_fin_pre(1)
            emit_fin_post(0)
            emit_fin_post(1)

    nc.compile()
    return nc


def kernel(mag_features, phase_features, noisy_mag, noisy_phase,
           mag_gamma, mag_beta, mag_W1, mag_b1, mag_W2, mag_b2,
           ph_gamma, ph_beta, ph_W1, ph_b1, ph_W2, ph_b2):
    if "nc" not in _cache:
        _cache["nc"] = _build()
    nc = _cache["nc"]

    mW1gT, mb1pT, mW2Tp, mb2c = _prep_branch(
        np.asarray(mag_gamma), np.asarray(mag_beta), np.asarray(mag_W1),
        np.asarray(mag_b1), np.asarray(mag_W2) * 0.5, np.asarray(mag_b2) * 0.5)
    pW1gT, pb1pT, pW2Tp, pb2c = _prep_branch(
        np.asarray(ph_gamma), np.asarray(ph_beta), np.asarray(ph_W1),
        np.asarray(ph_b1), np.asarray(ph_W2), np.asarray(ph_b2))

    shared = dict(
        w1gt_m=mW1gT, b1pt_m=mb1pT, w2tp_m=mW2Tp, b2c_m=mb2c,
        w1gt_p=pW1gT, b1pt_p=pb1pT, w2tp_p=pW2Tp, b2c_p=pb2c,
        ones_col=np.ones((128, 1), np.float32),
        ones_row=np.ones((1, 128), np.float32),
        halfpi=np.full((128, 1), np.pi / 2, np.float32),
    )
    mag_features = np.asarray(mag_features)
    phase_features = np.asarray(phase_features)
    noisy_mag_half = np.asarray(noisy_mag) * np.float32(0.5)
    noisy_phase = np.asarray(noisy_phase)

    in_maps = []
    for b in range(B):
        m = dict(shared)
        # [C, T, K] -> [C, K, T] k-major, contiguous per-band slices
        m["feat_m"] = np.ascontiguousarray(
            mag_features[b].transpose(0, 2, 1)).reshape(C, K * T)
        m["feat_p"] = np.ascontiguousarray(
            phase_features[b].transpose(0, 2, 1)).reshape(C, K * T)
        m["noisy_m"] = np.ascontiguousarray(noisy_mag_half[b])
        m["noisy_p"] = np.ascontiguousarray(noisy_phase[b])
        in_maps.append(m)

    import os
    trace = bool(os.environ.get("BASS_PROFILE"))
    res = run_bass_kernel_spmd(nc, in_maps, list(range(B)), trace=trace)
    _cache["last_result"] = res
    out = np.stack([res.results[b]["out"].view(np.complex64) for b in range(B)])
    return out



# revision 2
# speedup vs baseline: 1.0078x; 1.0078x over previous
"""Trainium2 Bass kernel for nn_DualBranchDecoder.

Dual-branch band-split decoder: per-band GroupNorm -> fc1(C=128->H=512)+tanh
-> per-band fc2(H->w_k) -> sigmoid mag mask / tanh phase offset -> complex out.

Sharding: data-parallel over batch B=8 across 8 NeuronCores (one sample per
core).

v2 design notes:
- Features ship as RAW fp16 (host cast, k-major).  The GroupNorm
  (x - mean) * inv normalize is folded into the fc1 activation:
  tanh(inv * (W1g @ x) + be) with per-partition scale = inv (broadcast) and
  bias be = b1p - inv*mean*S1, S1[h] = sum_c W1g[h, c].  This removes the
  DVE normalize pass entirely, halves feature DMA, and lets fc1 matmuls
  start straight off the DMA (no stats dependency before PE).
- Stats chains run per (quad, branch) one quad ahead; their two tiny PE
  ops (cross-partition sum, broadcast) are injected mid-way through the
  current quad's matmul stream.
- All activations (Tanh + Sin) are served by one act-function table set
  (silu_and_others) via a get_activation_tables patch, so there is no
  mid-kernel ACT_TABLE_LOAD thrash; a dummy tanh warms the single load at
  t~0.
- The mag sigmoid is computed as tanh (0.5s folded into host W2/b2, +1/2
  in the final mask multiply).  fin chunk 0 (f<128) is emitted as soon as
  quad 5 completes; only chunk 1 sits on the tail, processed in two
  column halves to overlap DVE/ACT/DMA.
"""
import sys
sys.path.insert(0, '/opt/trn_rl_repo')

import numpy as np

import concourse.bacc as bacc
import concourse.tile as tile
import concourse.mybir as mybir
from concourse.bass_utils import run_bass_kernel_spmd

F32 = mybir.dt.float32
FP16 = mybir.dt.float16
H1DT = FP16
W2DT = FP16
AF = mybir.ActivationFunctionType
ALU = mybir.AluOpType

# problem constants (hardcoded per contract)
B, C, T = 8, 128, 512
BANDS = [2] + [3] * 10 + [8] * 12 + [16] * 7 + [17]
K = len(BANDS)                      # 31
F = sum(BANDS)                      # 257
H = 4 * C                           # 512
NHC = H // 128                      # 4 h-chunks
EPS = 1e-5

OFFS = np.concatenate([[0], np.cumsum(BANDS)]).astype(int)   # band start freqs
WPADS = [w + (w & 1) for w in BANDS]
WOFFS = np.concatenate([[0], np.cumsum(WPADS)]).astype(int)
WPTOT = int(WOFFS[-1])

QUADS = [(4 * i, 4) for i in range(7)] + [(28, 3)]
NQ = len(QUADS)
MAGIC = float(1.5 * 2 ** 23)
INV2PI = float(1.0 / (2 * np.pi))
N2PI = float(-2 * np.pi)
PI = float(np.pi)
TE = T + 4                          # fin chunk-1 width (f=256 row folded in)

_cache = {}


def _patch_act_tables():
    """Make every activation resolve to the one table set that truly
    contains both tanh and sin (silu_and_others), so the kernel does a
    single ACT_TABLE_LOAD.  Only the chooser's view is patched; the
    emitted act_func_set_id still indexes the real act_info.json."""
    import concourse.hw_specs as hw_specs
    if getattr(bacc, "_act_tables_patched", False):
        return
    _orig = hw_specs.get_activation_tables

    def patched(arch):
        tabs = _orig(arch)
        return {name: (funcs if name == 'silu_and_others' else set())
                for name, funcs in tabs.items()}

    bacc.get_activation_tables = patched
    bacc._act_tables_patched = True


def _prep_branch(gamma, beta, W1, b1, W2, b2):
    """Host-side constant prep for one branch. W2/b2 must be pre-scaled by
    the caller if the branch folds sigmoid into tanh."""
    W1g = W1 * gamma[:, None, :]                      # [K, H, C]
    W1gT = np.ascontiguousarray(W1g.transpose(2, 0, 1).reshape(C, K * H))
    W1gT = W1gT.astype(np.float16)
    b1p = b1 + np.einsum('khc,kc->kh', W1, beta)      # [K, H]
    S1 = W1g.sum(axis=2)                              # [K, H]
    b1pT = np.zeros((128, K * NHC), np.float32)
    s1T = np.zeros((128, K * NHC), np.float32)
    for k in range(K):
        for hc in range(NHC):
            b1pT[:, k * NHC + hc] = b1p[k, hc * 128:(hc + 1) * 128]
            s1T[:, k * NHC + hc] = S1[k, hc * 128:(hc + 1) * 128]
    W2Tp = np.zeros((128, NHC * WPTOT), np.float32)
    for k in range(K):
        w, off, woff = BANDS[k], OFFS[k], WOFFS[k]
        for hc in range(NHC):
            W2Tp[:, hc * WPTOT + woff: hc * WPTOT + woff + w] = \
                W2[off:off + w, hc * 128:(hc + 1) * 128].T
    W2Tp = W2Tp.astype(np.float16)
    b2g = np.zeros((128, NQ), np.float32)
    for q, (k0, nb) in enumerate(QUADS):
        for r in range(nb):
            k = k0 + r
            b2g[32 * r:32 * r + BANDS[k], q] = b2[OFFS[k]:OFFS[k] + BANDS[k]]
    return W1gT, b1pT, s1T, W2Tp, b2g


def _build():
    _patch_act_tables()
    nc = bacc.Bacc("TRN2", target_bir_lowering=False)

    ins = {}
    for br in ("m", "p"):
        ins[f"feat_{br}"] = nc.dram_tensor(f"feat_{br}", [C, K * T], FP16,
                                           kind="ExternalInput")
        ins[f"w1gt_{br}"] = nc.dram_tensor(f"w1gt_{br}", [C, K * H], FP16,
                                           kind="ExternalInput")
        ins[f"b1pt_{br}"] = nc.dram_tensor(f"b1pt_{br}", [128, K * NHC], F32,
                                           kind="ExternalInput")
        ins[f"s1t_{br}"] = nc.dram_tensor(f"s1t_{br}", [128, K * NHC], F32,
                                          kind="ExternalInput")
        ins[f"w2tp_{br}"] = nc.dram_tensor(f"w2tp_{br}", [128, NHC * WPTOT], W2DT,
                                           kind="ExternalInput")
        ins[f"b2c_{br}"] = nc.dram_tensor(f"b2c_{br}", [128, NQ], F32,
                                          kind="ExternalInput")
        ins[f"noisy_{br}"] = nc.dram_tensor(f"noisy_{br}", [F, T], F32,
                                            kind="ExternalInput")
    ones_col_d = nc.dram_tensor("ones_col", [128, 1], F32, kind="ExternalInput")
    ones_row_d = nc.dram_tensor("ones_row", [1, 128], F32, kind="ExternalInput")
    halfpi_d = nc.dram_tensor("halfpi", [128, 1], F32, kind="ExternalInput")
    out_d = nc.dram_tensor("out", [F, 2 * T], F32, kind="ExternalOutput")

    with tile.TileContext(nc) as tc:
        with (
            tc.tile_pool(name="featk", bufs=6) as featk_pool,
            tc.tile_pool(name="w1t", bufs=6) as w1t_pool,
            tc.tile_pool(name="h1sb", bufs=3) as h1sb_pool,
            tc.tile_pool(name="band", bufs=4) as band_pool,
            tc.tile_pool(name="const", bufs=1) as const_pool,
            tc.tile_pool(name="statsb", bufs=2) as stats_pool,
            tc.tile_pool(name="fin", bufs=2) as fin_pool,
            tc.tile_pool(name="mainps", bufs=1, space="PSUM") as main_ps,
        ):
            # ---- critical-path first: quad-0 fetches before anything else ----
            st = {}     # per-quad pipeline state
            k0_0, nb_0 = QUADS[0]
            st[0] = {}
            st[0]["fq_m"] = featk_pool.tile([128, nb_0 * T], FP16, tag="featq",
                                            name="featq_m_0")
            nc.sync.dma_start(st[0]["fq_m"][:],
                              ins["feat_m"][:, k0_0 * T:(k0_0 + nb_0) * T])
            st[0]["wq_m"] = w1t_pool.tile([128, nb_0 * H], FP16, tag="w1q",
                                          name="w1q_m_0")
            nc.sync.dma_start(st[0]["wq_m"][:],
                              ins["w1gt_m"][:, k0_0 * H:(k0_0 + nb_0) * H])
            st[0]["fq_p"] = featk_pool.tile([128, nb_0 * T], FP16, tag="featq",
                                            name="featq_p_0")
            nc.sync.dma_start(st[0]["fq_p"][:],
                              ins["feat_p"][:, k0_0 * T:(k0_0 + nb_0) * T])
            st[0]["wq_p"] = w1t_pool.tile([128, nb_0 * H], FP16, tag="w1q",
                                          name="w1q_p_0")
            nc.sync.dma_start(st[0]["wq_p"][:],
                              ins["w1gt_p"][:, k0_0 * H:(k0_0 + nb_0) * H])

            # ---- constants ----
            ones_col = const_pool.tile([128, 1], F32)
            nc.sync.dma_start(ones_col[:], ones_col_d[:])
            ones_row = const_pool.tile([1, 128], F32)
            nc.sync.dma_start(ones_row[:], ones_row_d[:])
            halfpi = const_pool.tile([128, 1], F32)
            nc.sync.dma_start(halfpi[:], halfpi_d[:])

            cb = {}
            for br in ("m", "p"):
                b1pt = const_pool.tile([128, K * NHC], F32, tag=f"b1pt_{br}",
                                       name=f"b1pt_{br}")
                nc.sync.dma_start(b1pt[:], ins[f"b1pt_{br}"][:])
                s1t = const_pool.tile([128, K * NHC], F32, tag=f"s1t_{br}",
                                      name=f"s1t_{br}")
                nc.sync.dma_start(s1t[:], ins[f"s1t_{br}"][:])
                w2tp = const_pool.tile([128, NHC * WPTOT], W2DT, tag=f"w2tp_{br}",
                                       name=f"w2tp_{br}")
                nc.sync.dma_start(w2tp[:], ins[f"w2tp_{br}"][:])
                b2c = const_pool.tile([128, NQ], F32, tag=f"b2c_{br}",
                                      name=f"b2c_{br}")
                nc.sync.dma_start(b2c[:], ins[f"b2c_{br}"][:])
                cb[br] = (b1pt, s1t, w2tp, b2c)

            # noisy inputs prefetched into per-chunk tiles for the fin stage.
            # Chunk 1 is [128, T+4]: cols T.. hold the f=256 row reshaped as
            # [128, 4] (t = 4*p + c), so no third full-width pipeline exists.
            noisy = {}
            for br in ("m", "p"):
                n0 = const_pool.tile([128, T], F32, tag=f"noisy_{br}_0",
                                     name=f"noisy_{br}_0")
                nc.gpsimd.dma_start(n0[:], ins[f"noisy_{br}"][0:128, :])
                n1 = const_pool.tile([128, TE], F32, tag=f"noisy_{br}_1",
                                     name=f"noisy_{br}_1")
                nc.gpsimd.dma_start(n1[:, 0:T], ins[f"noisy_{br}"][128:256, :])
                nc.gpsimd.dma_start(n1[:, T:TE], ins[f"noisy_{br}"][256:257, :])
                noisy[br, 0] = n0
                noisy[br, 1] = n1

            # warm the single act table load right away (set covers tanh+sin)
            actwarm = stats_pool.tile([128, 1], F32, tag="actwarm",
                                      name="actwarm")
            nc.scalar.activation(actwarm[:], ones_col[:], AF.Tanh)

            # ---- PE warm-up: continuous matmul during DMA ramp-in ----
            for wi in range(16):
                wps = main_ps.tile([128, T], F32, tag="h1ps", bufs=5,
                                   name=f"warm_{wi}")
                nc.tensor.matmul(wps[:], st[0]["fq_m"][:, 0:128],
                                 st[0]["fq_m"][:, 0:T], start=True, stop=True)

            masks = {}
            for br in ("m", "p"):
                masks[br, 0] = const_pool.tile([128, T], F32, tag=f"mask_{br}_0",
                                               name=f"mask_{br}_0")
                masks[br, 1] = const_pool.tile([128, TE], F32, tag=f"mask_{br}_1",
                                               name=f"mask_{br}_1")

            # ---------------- pipeline stage emitters ----------------
            def do_dma(q):
                k0, nb = QUADS[q]
                s = st.setdefault(q, {})
                for br in ("m", "p"):
                    s[f"fq_{br}"] = featk_pool.tile([128, nb * T], FP16,
                                                    tag="featq",
                                                    name=f"featq_{br}_{q}")
                    nc.sync.dma_start(
                        s[f"fq_{br}"][:],
                        ins[f"feat_{br}"][:, k0 * T:(k0 + nb) * T])
                    s[f"wq_{br}"] = w1t_pool.tile([128, nb * H], FP16,
                                                  tag="w1q",
                                                  name=f"w1q_{br}_{q}")
                    nc.sync.dma_start(
                        s[f"wq_{br}"][:],
                        ins[f"w1gt_{br}"][:, k0 * H:(k0 + nb) * H])

            def do_front(q, br):
                """bn stats + per-partition (sum | sumsq) for one branch."""
                k0, nb = QUADS[q]
                s = st[q]
                st_q = stats_pool.tile([128, nb * 6], F32, tag="st_q",
                                       name=f"st_{br}_{q}")
                ag_q = stats_pool.tile([128, nb * 2], F32, tag="ag_q",
                                       name=f"ag_{br}_{q}")
                sums = stats_pool.tile([128, 2 * nb], F32, tag=f"sums_{br}",
                                       name=f"sums_{br}_{q}")
                tmp = stats_pool.tile([128, nb], F32, tag="tmp",
                                      name=f"tmp_{br}_{q}")
                fq = s[f"fq_{br}"]
                for r in range(nb):
                    nc.vector.bn_stats(st_q[:, r * 6:(r + 1) * 6],
                                       fq[:, r * T:(r + 1) * T])
                    nc.vector.bn_aggr(ag_q[:, r * 2:(r + 1) * 2],
                                      st_q[:, r * 6:(r + 1) * 6])
                ag3 = ag_q[:].rearrange("c (k two) -> c k two", two=2)
                nc.vector.tensor_copy(sums[:, 0:nb], ag3[:, :, 0])
                nc.vector.tensor_mul(tmp[:], ag3[:, :, 0], ag3[:, :, 0])
                nc.vector.tensor_add(sums[:, nb:2 * nb], tmp[:], ag3[:, :, 1])
                s[f"sums_{br}"] = sums

            def do_ps_s(q, br):
                """cross-partition reduction (PE, tiny)."""
                nb = QUADS[q][1]
                s = st[q]
                ps_s = main_ps.tile([1, 2 * nb], F32, tag="ps_s", bufs=1,
                                    name=f"ps_s_{br}_{q}")
                nc.tensor.matmul(ps_s[:], ones_col[:], s[f"sums_{br}"][:],
                                 start=True, stop=True)
                s[f"ps_s_{br}"] = ps_s

            def do_smid(q, br):
                """mean/var -> rsqrt via quake seed + 3 Newton (DVE), then
                pack invim = [inv | inv*mean]."""
                nb = QUADS[q][1]
                s = st[q]
                g = stats_pool.tile([1, 2 * nb], F32, tag="g",
                                    name=f"g_{br}_{q}")
                nc.vector.tensor_scalar_mul(g[:], s[f"ps_s_{br}"][:], 1.0 / C)
                gm2 = stats_pool.tile([1, nb], F32, tag="gm2",
                                      name=f"gm2_{br}_{q}")
                nc.vector.tensor_mul(gm2[:], g[:, 0:nb], g[:, 0:nb])
                gvar = stats_pool.tile([1, nb], F32, tag="gvar",
                                       name=f"gvar_{br}_{q}")
                nc.vector.tensor_sub(gvar[:], g[:, nb:2 * nb], gm2[:])
                vv = stats_pool.tile([1, nb], F32, tag="vv",
                                     name=f"vv_{br}_{q}")
                nc.vector.tensor_scalar_add(vv[:], gvar[:], EPS)
                I32 = mybir.dt.int32
                yy = stats_pool.tile([1, nb], F32, tag="yy",
                                     name=f"yy_{br}_{q}")
                nc.vector.tensor_scalar(yy[:].bitcast(I32), vv[:].bitcast(I32),
                                        1, -1, op0=ALU.arith_shift_right,
                                        op1=ALU.bitwise_xor)
                nc.vector.tensor_scalar_add(yy[:].bitcast(I32),
                                            yy[:].bitcast(I32), 0x5f3759e0)
                invim = stats_pool.tile([1, 2 * nb], F32, tag="invim",
                                        name=f"invim_{br}_{q}")
                tnr = stats_pool.tile([1, nb], F32, tag="tnr",
                                      name=f"tnr_{br}_{q}")
                for it in range(3):
                    nc.vector.tensor_mul(tnr[:], yy[:], yy[:])
                    nc.vector.tensor_mul(tnr[:], tnr[:], vv[:])
                    nc.vector.tensor_scalar(tnr[:], tnr[:], -0.5, 1.5,
                                            op0=ALU.mult, op1=ALU.add)
                    dst = yy[:] if it < 2 else invim[:, 0:nb]
                    nc.vector.tensor_mul(dst, yy[:], tnr[:])
                nc.vector.tensor_mul(invim[:, nb:2 * nb], invim[:, 0:nb],
                                     g[:, 0:nb])
                s[f"invim_{br}"] = invim

            def do_ps_b(q, br):
                """broadcast inv / inv*mean to all partitions (PE, tiny)."""
                nb = QUADS[q][1]
                s = st[q]
                ps_b = main_ps.tile([128, 2 * nb], F32, tag="ps_s", bufs=1,
                                    name=f"ps_b_{br}_{q}")
                nc.tensor.matmul(ps_b[:], ones_row[:], s[f"invim_{br}"][:],
                                 start=True, stop=True)
                s[f"ps_b_{br}"] = ps_b

            def do_sback(q, br):
                """bbq copy + per-band fc1 bias be = b1p - im*S1 (DVE)."""
                k0, nb = QUADS[q]
                s = st[q]
                b1pt, s1t = cb[br][0], cb[br][1]
                bbq = stats_pool.tile([128, 2 * nb], F32, tag=f"bbq_{br}",
                                      bufs=3, name=f"bbq_{br}_{q}")
                nc.vector.tensor_copy(bbq[:], s[f"ps_b_{br}"][:])
                be = stats_pool.tile([128, nb * NHC], F32, tag=f"be_{br}",
                                     bufs=3, name=f"be_{br}_{q}")
                for r in range(nb):
                    k = k0 + r
                    nc.vector.tensor_scalar(
                        be[:, r * NHC:(r + 1) * NHC],
                        s1t[:, k * NHC:(k + 1) * NHC],
                        bbq[:, nb + r:nb + r + 1], None, op0=ALU.mult)
                nc.vector.tensor_sub(be[:],
                                     b1pt[:, k0 * NHC:(k0 + nb) * NHC], be[:])
                s[f"bbq_{br}"] = bbq
                s[f"be_{br}"] = be

            def do_chain_dve(q, br):
                do_front(q, br)
                # ps_s must be injected on PE by the caller between front
                # and smid; smid stalls until it lands.

            def do_branch(q, br, hooks=()):
                """fc1 + fc2 + mask copy-out for one branch of one quad.

                hooks: list of (trigger, fn) with trigger in
                {'fc1_r2', 'fc1_end', 'fc2_r2', 'fc2_end'}."""
                hooks = dict(hooks)
                k0, nb = QUADS[q]
                s = st[q]
                b1pt, s1t, w2tp, b2c = cb[br]
                wq, fq = s[f"wq_{br}"], s[f"fq_{br}"]
                bbq, be = s[f"bbq_{br}"], s[f"be_{br}"]
                h1s = []
                for r in range(nb):
                    k = k0 + r
                    h1sb = h1sb_pool.tile([128, NHC * T], H1DT, bufs=6)
                    h1s.append(h1sb)
                    for hc in range(NHC):
                        h1ps = main_ps.tile([128, T], F32, tag="h1ps", bufs=5,
                                            name=f"h1ps_{br}_{k}_{hc}")
                        nc.tensor.matmul(
                            h1ps[:],
                            wq[:, (r * NHC + hc) * 128:(r * NHC + hc + 1) * 128],
                            fq[:, r * T:(r + 1) * T],
                            start=True, stop=True)
                        nc.scalar.activation(
                            h1sb[:, hc * T:(hc + 1) * T], h1ps[:],
                            AF.Tanh,
                            bias=be[:, r * NHC + hc:r * NHC + hc + 1],
                            scale=bbq[:, r:r + 1])
                    if r == 1 and 'fc1_r2' in hooks:
                        hooks['fc1_r2']()
                if 'fc1_end' in hooks:
                    hooks['fc1_end']()
                fc2g = main_ps.tile([128, T], F32, tag="fc2ps", bufs=2,
                                    name=f"fc2g_{br}_{q}")
                for r in range(nb):
                    k = k0 + r
                    wp, woff = WPADS[k], int(WOFFS[k])
                    for hc in range(NHC):
                        nc.tensor.matmul(
                            fc2g[32 * r:32 * r + wp, :],
                            w2tp[:, hc * WPTOT + woff: hc * WPTOT + woff + wp],
                            h1s[r][:, hc * T:(hc + 1) * T],
                            start=(hc == 0), stop=(hc == NHC - 1),
                            tile_position=(0, 32 * r))
                    if r == 1 and 'fc2_r2' in hooks:
                        hooks['fc2_r2']()
                if 'fc2_end' in hooks:
                    hooks['fc2_end']()
                grp_t = band_pool.tile([128, T], F32, tag="band")
                # mag: sigmoid(y+b2) = 0.5*(tanh(0.5*y+0.5*b2)+1); the 0.5s
                # live in host W2/b2 prep, the +1/2 in emit_fin.
                nc.scalar.activation(grp_t[:], fc2g[:], AF.Tanh,
                                     bias=b2c[:, q:q + 1])
                dma_eng = nc.sync if q == NQ - 1 else nc.gpsimd
                for r in range(nb):
                    k = k0 + r
                    w, off = BANDS[k], int(OFFS[k])
                    j0, r0 = off // 128, off % 128
                    if off + w <= (j0 + 1) * 128:
                        dma_eng.dma_start(masks[br, j0][r0:r0 + w, 0:T],
                                          grp_t[32 * r:32 * r + w, :])
                    else:
                        n1 = (j0 + 1) * 128 - off
                        dma_eng.dma_start(masks[br, j0][r0:128, 0:T],
                                          grp_t[32 * r:32 * r + n1, :])
                        if j0 == 0:
                            dma_eng.dma_start(
                                masks[br, 1][0:w - n1, 0:T],
                                grp_t[32 * r + n1:32 * r + w, :])
                        else:
                            # f=256 single row -> [128, 4] (t = 4*p + c)
                            dma_eng.dma_start(
                                masks[br, 1][:, T:TE],
                                grp_t[32 * r + n1:32 * r + w, :])

            fin_state = {}

            def emit_fin_pre(j, c0, c1):
                """DVE part of the final assembly for chunk j, cols c0:c1."""
                cols = c1 - c0
                mask_ap = masks["m", j][:, c0:c1]
                poff_ap = masks["p", j][:, c0:c1]
                nmag = noisy["m", j][:, c0:c1]      # pre-halved on host
                nph = noisy["p", j][:, c0:c1]
                tag = f"f{j}_{c0}"
                ang = fin_pool.tile([128, cols], F32, tag=f"ang{tag}")
                nc.vector.scalar_tensor_tensor(ang[:], poff_ap, PI, nph,
                                               op0=ALU.mult, op1=ALU.add)
                enh = fin_pool.tile([128, cols], F32, tag=f"enh{tag}")
                nc.vector.scalar_tensor_tensor(enh[:], mask_ap, 1.0, nmag,
                                               op0=ALU.add, op1=ALU.mult)
                # range-reduce ang to [-pi, pi) for the Sin table via
                # magic-number rounding
                t2 = fin_pool.tile([128, cols], F32, tag=f"t2{tag}")
                nc.vector.tensor_scalar(t2[:], ang[:], INV2PI, MAGIC,
                                        op0=ALU.mult, op1=ALU.add)
                m2pin = fin_pool.tile([128, cols], F32, tag=f"m2pin{tag}")
                nc.vector.tensor_scalar(m2pin[:], t2[:], MAGIC, N2PI,
                                        op0=ALU.subtract, op1=ALU.mult)
                nc.vector.tensor_add(m2pin[:], ang[:], m2pin[:])
                t2c = fin_pool.tile([128, cols], F32, tag=f"t2c{tag}")
                nc.vector.tensor_scalar(t2c[:], ang[:], INV2PI, 0.25,
                                        op0=ALU.mult, op1=ALU.add)
                nc.vector.tensor_scalar_add(t2c[:], t2c[:], MAGIC)
                m2pinc = fin_pool.tile([128, cols], F32, tag=f"m2pinc{tag}")
                nc.vector.tensor_scalar(m2pinc[:], t2c[:], MAGIC, N2PI,
                                        op0=ALU.subtract, op1=ALU.mult)
                nc.vector.tensor_add(m2pinc[:], ang[:], m2pinc[:])
                fin_state[j, c0] = (cols, enh, m2pin, m2pinc)

            def emit_fin_post(j, c0, dma_engs):
                """Sin/cos + complex assembly + output DMA for chunk j cols
                c0.., split across dma_engs."""
                cols, enh, m2pin, m2pinc = fin_state[j, c0]
                tag = f"f{j}_{c0}"
                sn = fin_pool.tile([128, cols], F32, tag=f"sn{tag}")
                nc.scalar.activation(sn[:], m2pin[:], AF.Sin)
                cn = fin_pool.tile([128, cols], F32, tag=f"cn{tag}")
                nc.scalar.activation(cn[:], m2pinc[:], AF.Sin, bias=halfpi[:])
                ot = fin_pool.tile([128, 2 * cols], F32, tag=f"ot{tag}")
                ot2 = ot[:].rearrange("p (t two) -> p t two", two=2)
                nc.vector.tensor_mul(ot2[:, :, 0], enh[:], cn[:])
                nc.vector.tensor_mul(ot2[:, :, 1], enh[:], sn[:])
                # output cols [2*c0 : 2*min(c1,T)] of chunk j
                cend = min(c0 + cols, T)
                wid = 2 * (cend - c0)
                nsp = len(dma_engs)
                step = (wid + nsp - 1) // nsp
                step += step & 1
                for i, eng in enumerate(dma_engs):
                    a, b = i * step, min((i + 1) * step, wid)
                    if a >= b:
                        continue
                    eng.dma_start(
                        out_d[j * 128:(j + 1) * 128, 2 * c0 + a:2 * c0 + b],
                        ot[:, a:b])
                if c0 + cols > T:      # folded f=256 row
                    nc.sync.dma_start(out_d[256:257, :],
                                      ot[:, 2 * (T - c0):2 * (TE - c0)])

            # ---------------- software-pipelined main loop ----------------
            # prologue: quad-0 stats (m first, then p), quad-1 DMA
            do_dma(1)
            do_front(0, "m")            # DVE
            do_ps_s(0, "m")             # PE (after warmup in queue)
            do_smid(0, "m")             # DVE
            do_ps_b(0, "m")             # PE
            do_sback(0, "m")            # DVE
            do_front(0, "p")            # DVE

            for q in range(NQ):
                if q + 2 < NQ:
                    do_dma(q + 2)
                nxt = q + 1 if q + 1 < NQ else None

                # DVE stats for next quad's m branch start now
                if nxt is not None:
                    do_front(nxt, "m")

                def mk(fns):
                    def run():
                        for f in fns:
                            f()
                    return run

                # m-branch hooks:
                #  fc1_r2: PE ps_s for this quad's p chain (q==0) is done in
                #          prologue; here inject next-quad m ps_s.
                #  fc1_end: next-quad m ps_b + DVE sback + p front
                m_hooks = {}
                p_hooks = {}
                if q == 0:
                    # quad-0 p chain still needs its PE bounces
                    m_hooks['fc1_r2'] = mk([lambda: do_ps_s(0, "p"),
                                            lambda: do_smid(0, "p")])
                    m_hooks['fc1_end'] = mk([lambda: do_ps_b(0, "p"),
                                             lambda: do_sback(0, "p")])
                    if nxt is not None:
                        m_hooks['fc2_r2'] = mk(
                            [lambda n=nxt: do_ps_s(n, "m"),
                             lambda n=nxt: do_smid(n, "m")])
                        m_hooks['fc2_end'] = mk(
                            [lambda n=nxt: do_ps_b(n, "m"),
                             lambda n=nxt: do_sback(n, "m"),
                             lambda n=nxt: do_front(n, "p")])
                        p_hooks['fc1_r2'] = mk(
                            [lambda n=nxt: do_ps_s(n, "p"),
                             lambda n=nxt: do_smid(n, "p")])
                        p_hooks['fc1_end'] = mk(
                            [lambda n=nxt: do_ps_b(n, "p"),
                             lambda n=nxt: do_sback(n, "p")])
                else:
                    if nxt is not None:
                        m_hooks['fc1_r2'] = mk(
                            [lambda n=nxt: do_ps_s(n, "m"),
                             lambda n=nxt: do_smid(n, "m")])
                        m_hooks['fc1_end'] = mk(
                            [lambda n=nxt: do_ps_b(n, "m"),
                             lambda n=nxt: do_sback(n, "m"),
                             lambda n=nxt: do_front(n, "p")])
                        m_hooks['fc2_end'] = mk(
                            [lambda n=nxt: do_ps_s(n, "p"),
                             lambda n=nxt: do_smid(n, "p")])
                        p_hooks['fc1_r2'] = mk(
                            [lambda n=nxt: do_ps_b(n, "p"),
                             lambda n=nxt: do_sback(n, "p")])

                do_branch(q, "m", hooks=m_hooks)
                do_branch(q, "p", hooks=p_hooks)

                if q == 5:
                    # bands 0..22 (f 0..127) complete for both branches
                    emit_fin_pre(0, 0, T)
                if q == 6:
                    emit_fin_post(0, 0, (nc.gpsimd,))

            # tail: chunk 1 in two column halves to overlap DVE/ACT/DMA
            HALF = 260                # even split point (<= T)
            emit_fin_pre(1, 0, HALF)
            emit_fin_post(1, 0, (nc.sync, nc.scalar))
            emit_fin_pre(1, HALF, TE)
            emit_fin_post(1, HALF, (nc.sync, nc.scalar))

    nc.compile()
    return nc


def kernel(mag_features, phase_features, noisy_mag, noisy_phase,
           mag_gamma, mag_beta, mag_W1, mag_b1, mag_W2, mag_b2,
           ph_gamma, ph_beta, ph_W1, ph_b1, ph_W2, ph_b2):
    if "nc" not in _cache:
        _cache["nc"] = _build()
    nc = _cache["nc"]

    mW1gT, mb1pT, ms1T, mW2Tp, mb2c = _prep_branch(
        np.asarray(mag_gamma), np.asarray(mag_beta), np.asarray(mag_W1),
        np.asarray(mag_b1), np.asarray(mag_W2) * 0.5, np.asarray(mag_b2) * 0.5)
    pW1gT, pb1pT, ps1T, pW2Tp, pb2c = _prep_branch(
        np.asarray(ph_gamma), np.asarray(ph_beta), np.asarray(ph_W1),
        np.asarray(ph_b1), np.asarray(ph_W2), np.asarray(ph_b2))

    shared = dict(
        w1gt_m=mW1gT, b1pt_m=mb1pT, s1t_m=ms1T, w2tp_m=mW2Tp, b2c_m=mb2c,
        w1gt_p=pW1gT, b1pt_p=pb1pT, s1t_p=ps1T, w2tp_p=pW2Tp, b2c_p=pb2c,
        ones_col=np.ones((128, 1), np.float32),
        ones_row=np.ones((1, 128), np.float32),
        halfpi=np.full((128, 1), np.pi / 2, np.float32),
    )
    mag_features = np.asarray(mag_features)
    phase_features = np.asarray(phase_features)
    noisy_mag_half = np.asarray(noisy_mag) * np.float32(0.5)
    noisy_phase = np.asarray(noisy_phase)

    in_maps = []
    for b in range(B):
        m = dict(shared)
        # [C, T, K] -> [C, K, T] k-major, contiguous per-band slices; raw
        # (un-normalized) fp16
        m["feat_m"] = np.ascontiguousarray(
            mag_features[b].transpose(0, 2, 1)).reshape(C, K * T).astype(
                np.float16)
        m["feat_p"] = np.ascontiguousarray(
            phase_features[b].transpose(0, 2, 1)).reshape(C, K * T).astype(
                np.float16)
        m["noisy_m"] = np.ascontiguousarray(noisy_mag_half[b])
        m["noisy_p"] = np.ascontiguousarray(noisy_phase[b])
        in_maps.append(m)

    import os
    trace = bool(os.environ.get("BASS_PROFILE"))
    res = run_bass_kernel_spmd(nc, in_maps, list(range(B)), trace=trace)
    _cache["last_result"] = res
    out = np.stack([res.results[b]["out"].view(np.complex64) for b in range(B)])
    return out
